# revision 18
# baseline (speedup 1.0000x reference)
"""Trainium2 Bass kernel for nn_DEQTransformerLM (Transformer-XL style DEQ layer).

Sharding: local-window attention (window 256, qlen 512, mlen 512) makes the
layer fully token-parallel: 8 cores = 4 batches x 2 query-halves of 256
queries. Each core runs qkv projections, relative attention, o-proj+LN and the
FF block for its 256 tokens. No collectives; weights replicated (fp16).

Attention inner loop (per head h, query-chunk c of 128 queries):
  - BD_raw rows are written to DRAM scratch as NEG-padded 513-wide rows and
    read back as a 512-stride view, landing BD_raw[q, r] at score column
    q+1+r with NEG (additive mask) elsewhere - rel_shift + mask in one DMA.
  - score = AC (PSUM) + shifted-BD; EXP on the scalar engine emits the
    softmax denominator per query via accum_out for free; denominators for
    the 4 (c, head) units of a pair batch into one [128,4] reciprocal
    (partition-parallel, fast); probs are normalized pre-transpose on the
    otherwise-idle gpsimd engine.
  - normalized fp16 probs are transposed on the PE (fp16 identity, 1
    cycle/row) into one [128,384] fp16 PSUM tile per unit, then a single
    vector/scalar cast moves them to SBUF for the AV matmuls. No ones/bcast
    matmuls, no [1,256] reciprocals, no fp32 transposes.

Matmul operands fp16 (peak PE rate), accumulation fp32. Layernorm broadcasts
stats to 128 partitions BEFORE sqrt/reciprocal so those run wide.
"""

import contextlib
import os

import numpy as np

import concourse.bacc as bacc
import concourse.mybir as mybir
import concourse.tile as tile
from concourse.bass_utils import run_bass_kernel_spmd
from concourse.masks import make_identity

FP16 = mybir.dt.float16
FP32 = mybir.dt.float32
AF = mybir.ActivationFunctionType
ALU = mybir.AluOpType

D = 1024          # d_model
NH = 16           # heads
DH = 64           # d_head
QC = 256          # queries per core
KW = 512          # key window per core
RW = 256          # relative positions used (cols 768:1024 of r_head_k)
DI = 4096         # d_inner
NC_ = D // 128    # 8 d_model chunks
NI = DI // 128    # 32 d_inner chunks

SCR_ROW = 513     # padded scratch row width (BD rel-shift trick)
SCR_HEAD = QC * SCR_ROW
NEG = -30000.0    # additive mask value (fp16-safe)

_CACHE = {}
PHASE = int(os.environ.get("DEQ_PHASE", "5"))
L_BD = int(os.environ.get("DEQ_L_BD", "2"))    # bd stage lookahead (pairs)


def build_nc():
    nc = bacc.Bacc("TRN2", target_bir_lowering=False, debug=False)
    P = nc.declare_dram_parameter

    # weights pre-blocked on host as [mc, 128, kc*128] (lhsT column blocks,
    # partition-major) so each block load is one contiguous 2KB-row DMA
    wq_b = P("wq_b", [NC_, 128, D], FP16, isOutput=False)
    wk_b = P("wk_b", [NC_, 128, D], FP16, isOutput=False)
    rw_b = P("rw_b", [NC_, 128, D], FP16, isOutput=False)
    ow_b = P("ow_b", [NC_, 128, D], FP16, isOutput=False)
    ff1w_b = P("ff1w_b", [NI, 128, D], FP16, isOutput=False)
    ff2w_b = P("ff2w_b", [NC_, 128, DI], FP16, isOutput=False)
    wv_p = P("wv_p", [128, NC_, D], FP16, isOutput=False)   # Wv.T row-chunks

    # activations pre-arranged to [128, chunk, cols]
    xq = P("xq", [128, NC_, QC], FP16, isOutput=False)
    xkv = P("xkv", [128, NC_, KW], FP16, isOutput=False)
    uqw = P("uqw", [128, NC_, QC], FP16, isOutput=False)
    uqr = P("uqr", [128, NC_, QC], FP16, isOutput=False)
    uk = P("uk", [128, NC_, KW], FP16, isOutput=False)
    uvt = P("uvt", [128, 4, D], FP16, isOutput=False)
    pos = P("pos", [128, NC_, RW], FP16, isOutput=False)
    ob = P("ob", [128, NC_], FP32, isOutput=False)
    f1b = P("f1b", [128, NI], FP32, isOutput=False)
    f2b = P("f2b", [128, NC_], FP32, isOutput=False)

    out = P("out", [D, QC], FP16, isOutput=True)

    scr = nc.dram_tensor("scr", [NH * SCR_HEAD], FP16)

    with contextlib.ExitStack() as _es:
        tc = _es.enter_context(tile.TileContext(nc))
        pool = lambda **kw: _es.enter_context(tc.tile_pool(**kw))
        single = pool(name="single", bufs=1)
        wblk_pool = pool(name="wblk", bufs=8)
        ff1_pool = pool(name="ff1p", bufs=6)
        ff2_pool = pool(name="ff2p", bufs=2)
        bdpad_pool = pool(name="bdpad", bufs=4)
        bdm_pool = pool(name="bdm", bufs=12)
        sc_pool = pool(name="scp", bufs=4)
        pr_pool = pool(name="prp", bufs=4)
        prn_pool = pool(name="prn", bufs=6)
        den_pool = pool(name="den", bufs=3)
        pt_pool = pool(name="ptp", bufs=8)
        rows_pool = pool(name="rows", bufs=1)
        bc_pool = pool(name="bcp", bufs=1)
        out_pool = pool(name="outp", bufs=4)
        mm_ps = pool(name="mm_ps", bufs=2, space="PSUM")
        ac_ps_pool = pool(name="ac_ps", bufs=2, space="PSUM")
        bd_ps_pool = pool(name="bd_ps", bufs=1, space="PSUM")
        tp_ps_pool = pool(name="tp_ps", bufs=2, space="PSUM")
        av_ps_pool = pool(name="av_ps", bufs=1, space="PSUM")
        if True:
            # -------- resident tiles (loads split per chunk for fine deps) ---
            xkv_sb = single.tile([128, NC_, KW], FP16, tag="xkv")
            uk_sb = single.tile([128, NC_, KW], FP16, tag="uk")
            xq_sb = single.tile([128, NC_, QC], FP16, tag="xq")
            uqw_sb = single.tile([128, NC_, QC], FP16, tag="uqw")
            uqr_sb = single.tile([128, NC_, QC], FP16, tag="uqr")
            pos_sb = single.tile([128, NC_, RW], FP16, tag="pos")
            wv_sb = single.tile([128, NC_, D], FP16, tag="wv")
            uvt_sb = single.tile([128, 4, D], FP16, tag="uvt")

            def wload(param, mc):
                t = wblk_pool.tile([128, D], FP16, tag="wblk")
                nc.sync.dma_start(out=t, in_=param[:][mc])
                return t

            # interleave xkv chunks with wk blocks so K proj starts early
            wk_tiles = []
            for kc in range(NC_):
                nc.sync.dma_start(out=xkv_sb[:, kc, :], in_=xkv[:][:, kc, :])
                wk_tiles.append(wload(wk_b, kc))
                nc.scalar.dma_start(out=uk_sb[:, kc, :], in_=uk[:][:, kc, :])
            for kc in range(NC_):
                nc.scalar.dma_start(out=xq_sb[:, kc, :], in_=xq[:][:, kc, :])
                nc.scalar.dma_start(out=uqw_sb[:, kc, :], in_=uqw[:][:, kc, :])
                nc.scalar.dma_start(out=uqr_sb[:, kc, :], in_=uqr[:][:, kc, :])
                nc.scalar.dma_start(out=pos_sb[:, kc, :], in_=pos[:][:, kc, :])
            for kc in range(NC_):
                nc.scalar.dma_start(out=wv_sb[:, kc, :], in_=wv_p[:][:, kc, :])
            for jb in range(4):
                nc.scalar.dma_start(out=uvt_sb[:, jb, :], in_=uvt[:][:, jb, :])

            ones_k = single.tile([128, 1], FP16, tag="ones_k")
            nc.vector.memset(ones_k, 1.0)
            ones_m = single.tile([1, 128], FP16, tag="ones_m")
            nc.vector.memset(ones_m, 1.0)
            negc_sb = single.tile([128, 1], FP32, tag="negc")
            nc.vector.memset(negc_sb, -12.5)
            eps_row = single.tile([1, QC], FP32, tag="eps_row")
            nc.vector.memset(eps_row, 1e-5)
            ident = single.tile([128, 128], FP16, tag="ident")
            make_identity(nc, ident)

            ob_sb = single.tile([128, NC_], FP32, tag="ob")
            nc.gpsimd.dma_start(out=ob_sb, in_=ob[:])
            f1b_sb = single.tile([128, NI], FP32, tag="f1b")
            nc.gpsimd.dma_start(out=f1b_sb, in_=f1b[:])
            f2b_sb = single.tile([128, NC_], FP32, tag="f2b")
            nc.gpsimd.dma_start(out=f2b_sb, in_=f2b[:])

            k_sb = single.tile([128, NC_, KW], FP16, tag="k_sb")
            vT_sb = single.tile([128, 4, D], FP16, tag="vT_sb")
            rwq_sb = single.tile([128, NC_, QC], FP16, tag="rwq")
            rrq_sb = single.tile([128, NC_, QC], FP16, tag="rrq")
            r_sb = single.tile([128, NC_, RW], FP16, tag="r_sb")
            attn_sb = single.tile([128, NC_, QC], FP16, tag="attn")
            x_sb = single.tile([128, NC_, QC], FP16, tag="x_sb")
            h_sb = single.tile([128, NI, QC], FP16, tag="h_sb")

            # ---------------- K projection ----------------
            for mc in range(NC_ if PHASE >= 1 else 0):
                wt = wk_tiles[mc]
                ps = mm_ps.tile([128, KW], FP32, tag="mm")
                for kc in range(NC_):
                    nc.tensor.matmul(ps, wt[:, 128 * kc:128 * (kc + 1)],
                                     xkv_sb[:, kc, :],
                                     start=(kc == 0), stop=(kc == NC_ - 1))
                nc.vector.tensor_tensor(out=k_sb[:, mc, :], in0=ps,
                                        in1=uk_sb[:, mc, :], op=ALU.add)

            # ---------------- Q projection ----------------
            for mc in range(NC_ if PHASE >= 1 else 0):
                wt = wload(wq_b, mc)
                ps = mm_ps.tile([128, QC], FP32, tag="mm")
                for kc in range(NC_):
                    nc.tensor.matmul(ps, wt[:, 128 * kc:128 * (kc + 1)],
                                     xq_sb[:, kc, :],
                                     start=(kc == 0), stop=(kc == NC_ - 1))
                nc.vector.tensor_tensor(out=rwq_sb[:, mc, :], in0=ps,
                                        in1=uqw_sb[:, mc, :], op=ALU.add)
                nc.vector.tensor_tensor(out=rrq_sb[:, mc, :], in0=ps,
                                        in1=uqr_sb[:, mc, :], op=ALU.add)

            # ---------------- R projection ----------------
            for mc in range(NC_ if PHASE >= 1 else 0):
                wt = wload(rw_b, mc)
                ps = mm_ps.tile([128, RW], FP32, tag="mm")
                for kc in range(NC_):
                    nc.tensor.matmul(ps, wt[:, 128 * kc:128 * (kc + 1)],
                                     pos_sb[:, kc, :],
                                     start=(kc == 0), stop=(kc == NC_ - 1))
                nc.vector.tensor_copy(r_sb[:, mc, :], ps)

            # ---------------- V projection (transposed output) --------------
            for jb in range(4 if PHASE >= 1 else 0):
                for dh in range(2):
                    ps = mm_ps.tile([128, 512], FP32, tag="mm")
                    for kc in range(NC_):
                        nc.tensor.matmul(
                            ps,
                            xkv_sb[:, kc, 128 * jb:128 * (jb + 1)],
                            wv_sb[:, kc, 512 * dh:512 * (dh + 1)],
                            start=(kc == 0), stop=(kc == NC_ - 1))
                    nc.vector.tensor_tensor(
                        out=vT_sb[:, jb, 512 * dh:512 * (dh + 1)], in0=ps,
                        in1=uvt_sb[:, jb, 512 * dh:512 * (dh + 1)], op=ALU.add)

            # ------- attention, software-pipelined over head PAIRS ----------
            # pair pi = heads (2*pi, 2*pi+1): even head in partition rows
            # 0:64, odd in 64:128 of d-chunk hc=pi.
            scrap = scr[:]
            bdm_tiles = {}   # pi -> {(c, sub): shifted-BD tile [128, 384]}
            pt_tiles = {}    # pi -> {(c, sub): transposed probs [128, 384]}

            def bd_stage(pi):
                # BD matmuls + padded scratch writes + shifted read issue
                hc = pi
                bdm_tiles[pi] = {}
                for c in range(2):
                    for sub in range(2):
                        h = 2 * pi + sub
                        hsl = slice(64 * sub, 64 * sub + 64)
                        bdp = bd_ps_pool.tile([128, RW], FP32, tag="bd")
                        nc.tensor.matmul(
                            bdp, rrq_sb[hsl, hc, 128 * c:128 * (c + 1)],
                            r_sb[hsl, hc, :], start=True, stop=True)
                        pad = bdpad_pool.tile([128, SCR_ROW], FP16, tag="pad")
                        if pi == 0:
                            nc.gpsimd.memset(pad, NEG)
                        if sub == 0:
                            nc.vector.tensor_copy(pad[:, 1:257], bdp)
                        else:
                            nc.scalar.copy(pad[:, 1:257], bdp)
                        wbase = h * SCR_HEAD + SCR_ROW * 128 * c
                        nc.sync.dma_start(
                            out=scrap[wbase:wbase + 128 * SCR_ROW].rearrange(
                                "(p f) -> p f", f=SCR_ROW),
                            in_=pad)
                        rbase = h * SCR_HEAD + KW * 128 * c + 128 * c
                        bdm = bdm_pool.tile([128, 384], FP16, tag="bdm")
                        nc.scalar.dma_start(
                            out=bdm,
                            in_=scrap[rbase:rbase + 128 * KW].rearrange(
                                "(p f) -> p f", f=KW)[:, 0:384])
                        bdm_tiles[pi][(c, sub)] = bdm

            def sm_stage(pi):
                hc = pi
                pt_tiles[pi] = {}
                den4 = den_pool.tile([128, 4], FP32, tag="den")
                prns = {}
                for u, (c, sub) in enumerate(
                        (c, s) for c in range(2) for s in range(2)):
                    hsl = slice(64 * sub, 64 * sub + 64)
                    acp = ac_ps_pool.tile([128, 384], FP32, tag="ac")
                    nc.tensor.matmul(
                        acp, rwq_sb[hsl, hc, 128 * c:128 * (c + 1)],
                        k_sb[hsl, hc, 128 * c:128 * c + 384],
                        start=True, stop=True)
                    sc = sc_pool.tile([128, 384], FP16, tag="sc")
                    nc.vector.tensor_tensor(
                        out=sc, in0=acp, in1=bdm_tiles[pi][(c, sub)],
                        op=ALU.add)
                    pr = pr_pool.tile([128, 384], FP16, tag="pr")
                    nc.scalar.activation(out=pr, in_=sc, func=AF.Exp,
                                         scale=0.125, bias=negc_sb,
                                         accum_out=den4[:, u:u + 1])
                    prns[(c, sub)] = pr
                nc.vector.reciprocal(out=den4, in_=den4)
                for u, (c, sub) in enumerate(
                        (c, s) for c in range(2) for s in range(2)):
                    prn = prn_pool.tile([128, 384], FP16, tag="prn")
                    nc.gpsimd.tensor_scalar_mul(prn, prns[(c, sub)],
                                                den4[:, u:u + 1])
                    tp = tp_ps_pool.tile([128, 384], FP16, tag="tp")
                    for t in range(3):
                        nc.tensor.transpose(
                            tp[:, 128 * t:128 * (t + 1)],
                            prn[:, 128 * t:128 * (t + 1)], ident)
                    pt = pt_pool.tile([128, 384], FP16, tag="pt")
                    if u % 2 == 0:
                        nc.vector.tensor_copy(pt, tp)
                    else:
                        nc.scalar.copy(pt, tp)
                    pt_tiles[pi][(c, sub)] = pt
                del bdm_tiles[pi]

            def av_stage(pi):
                hc = pi
                for sub in range(2):
                    h = 2 * pi + sub
                    avp = av_ps_pool.tile([64, 256], FP32, tag="av")
                    for c in range(2):
                        ptt = pt_tiles[pi][(c, sub)]
                        for t in range(3):
                            kb = t + c
                            nc.tensor.matmul(
                                avp[:, 128 * c:128 * (c + 1)],
                                vT_sb[:, kb, 64 * h:64 * h + 64],
                                ptt[:, 128 * t:128 * (t + 1)],
                                start=(t == 0), stop=(t == 2))
                    nc.vector.tensor_copy(
                        attn_sb[64 * sub:64 * sub + 64, hc, :], avp)
                del pt_tiles[pi]

            NP_ = NH // 2
            if PHASE >= 2:
                for pi in range(min(L_BD, NP_)):
                    bd_stage(pi)
                sm_stage(0)
                for pi in range(NP_):
                    if pi + L_BD < NP_:
                        bd_stage(pi + L_BD)
                    if pi + 1 < NP_:
                        sm_stage(pi + 1)
                    av_stage(pi)

            # ---------------- O projection + LN1 ----------------
            # stack rows: cols 0:256 = attn_out + residual, 256:512 = square
            stack1 = single.tile([128, NC_, 512], FP16, tag="stack")
            for mc in range(NC_ if PHASE >= 3 else 0):
                wt = wload(ow_b, mc)
                ps = mm_ps.tile([128, QC], FP32, tag="mm")
                for kc in range(NC_):
                    nc.tensor.matmul(ps, wt[:, 128 * kc:128 * (kc + 1)],
                                     attn_sb[:, kc, :],
                                     start=(kc == 0), stop=(kc == NC_ - 1))
                nc.vector.scalar_tensor_tensor(
                    out=stack1[:, mc, 0:256], in0=ps, scalar=ob_sb[:, mc:mc + 1],
                    in1=xq_sb[:, mc, :], op0=ALU.add, op1=ALU.add)
                nc.vector.tensor_tensor(
                    out=stack1[:, mc, 256:512], in0=stack1[:, mc, 0:256],
                    in1=stack1[:, mc, 0:256], op=ALU.mult)

            def layernorm(stack, xout):
                # red [1,512] = (sum x | sum x^2); broadcast FIRST, then all
                # per-token math runs on 128 partitions.
                redt = mm_ps.tile([128, 512], FP32, tag="mm")
                red = redt[0:1, :]
                for mc in range(NC_):
                    nc.tensor.matmul(red, ones_k, stack[:, mc, :],
                                     start=(mc == 0), stop=(mc == NC_ - 1))
                row = rows_pool.tile([1, 512], FP32, tag="row")
                nc.vector.tensor_scalar_mul(row, red, 1.0 / D)  # mean | E[x^2]
                msq = rows_pool.tile([1, QC], FP32, tag="msq")
                nc.vector.tensor_tensor(out=msq, in0=row[0:1, 0:256],
                                        in1=row[0:1, 0:256], op=ALU.mult)
                # var + eps in cols 256:512
                nc.vector.tensor_tensor(out=row[0:1, 256:512],
                                        in0=row[0:1, 256:512], in1=msq,
                                        op=ALU.subtract)
                nc.vector.tensor_tensor(out=row[0:1, 256:512],
                                        in0=row[0:1, 256:512], in1=eps_row,
                                        op=ALU.add)
                row16 = rows_pool.tile([1, 512], FP16, tag="row16")
                nc.vector.tensor_copy(row16, row)
                bps = mm_ps.tile([128, 512], FP32, tag="mm")
                nc.tensor.matmul(bps, ones_m, row16, start=True, stop=True)
                std = bc_pool.tile([128, QC], FP32, tag="std")
                nc.scalar.activation(out=std, in_=bps[:, 256:512],
                                     func=AF.Sqrt)
                nc.vector.reciprocal(out=std, in_=std)
                mean = bc_pool.tile([128, QC], FP16, tag="mean")
                nc.vector.tensor_copy(mean, bps[:, 0:256])
                rstd16 = bc_pool.tile([128, QC], FP16, tag="rstd16")
                nc.vector.tensor_copy(rstd16, std)
                for mc in range(NC_):
                    xo = xout(mc)
                    nc.vector.tensor_tensor(
                        out=xo, in0=stack[:, mc, 0:256],
                        in1=mean, op=ALU.subtract)
                    nc.vector.tensor_tensor(out=xo, in0=xo, in1=rstd16,
                                            op=ALU.mult)

            if PHASE >= 3:
                layernorm(stack1, lambda mc: x_sb[:, mc, :])

            # ---------------- FF1 ----------------
            for mc in range(NI if PHASE >= 4 else 0):
                wt = ff1_pool.tile([128, D], FP16, tag="ff1")
                (nc.sync if mc % 2 == 0 else nc.scalar).dma_start(
                    out=wt, in_=ff1w_b[:][mc])
                ps = mm_ps.tile([128, QC], FP32, tag="mm")
                for kc in range(NC_):
                    nc.tensor.matmul(ps, wt[:, 128 * kc:128 * (kc + 1)],
                                     x_sb[:, kc, :],
                                     start=(kc == 0), stop=(kc == NC_ - 1))
                nc.scalar.activation(out=h_sb[:, mc, :], in_=ps, func=AF.Relu,
                                     bias=f1b_sb[:, mc:mc + 1], scale=1.0)

            # ---------------- FF2 + LN2 + output ----------------
            stack2 = single.tile([128, NC_, 512], FP16, tag="stack2")
            for mc in range(NC_ if PHASE >= 5 else 0):
                wt = ff2_pool.tile([128, DI], FP16, tag="ff2")
                (nc.sync if mc % 2 == 0 else nc.scalar).dma_start(
                    out=wt, in_=ff2w_b[:][mc])
                ps = mm_ps.tile([128, QC], FP32, tag="mm")
                for kc in range(NI):
                    nc.tensor.matmul(ps, wt[:, 128 * kc:128 * (kc + 1)],
                                     h_sb[:, kc, :],
                                     start=(kc == 0), stop=(kc == NI - 1))
                nc.vector.scalar_tensor_tensor(
                    out=stack2[:, mc, 0:256], in0=ps, scalar=f2b_sb[:, mc:mc + 1],
                    in1=x_sb[:, mc, :], op0=ALU.add, op1=ALU.add)
                nc.vector.tensor_tensor(
                    out=stack2[:, mc, 256:512], in0=stack2[:, mc, 0:256],
                    in1=stack2[:, mc, 0:256], op=ALU.mult)

            out_tiles = {}

            def out_tile(mc):
                t = out_pool.tile([128, QC], FP16, tag="out")
                out_tiles[mc] = t
                return t

            if PHASE >= 5:
                layernorm(stack2, out_tile)
            else:
                for mc in range(NC_):
                    t = out_tile(mc)
                    nc.vector.memset(t, 0.0)
            for mc in range(NC_):
                (nc.sync if mc % 2 == 0 else nc.scalar).dma_start(
                    out=out[:][128 * mc:128 * (mc + 1), :], in_=out_tiles[mc])

    nc.compile()
    return nc

def _chunked(a, n):
    # [n*128, w] -> [128, n, w] partition-major
    w = a.shape[1]
    return np.ascontiguousarray(a.reshape(n, 128, w).transpose(1, 0, 2))


def _blocked(wt, nmc, nkc):
    # wt [K, M] (transposed weight) -> [mc, 128, kc*128] lhsT column blocks
    return np.ascontiguousarray(
        wt.reshape(nkc, 128, nmc, 128).transpose(2, 1, 0, 3).reshape(
            nmc, 128, nkc * 128))


def _prep_inputs(z1ss, uss, z0, pos_emb, qkv_w, r_w, r_w_bias, r_r_bias, o_w,
                 o_b, ff1_w, ff1_b, ff2_w, ff2_b):
    f16 = np.float16
    c = np.ascontiguousarray

    wq_t = qkv_w[0:D].T.astype(f16)
    wk_t = qkv_w[D:2 * D].T.astype(f16)
    wv_t = qkv_w[2 * D:3 * D].T.astype(f16)
    shared = dict(
        wq_b=_blocked(wq_t, NC_, NC_), wk_b=_blocked(wk_t, NC_, NC_),
        rw_b=_blocked(r_w.T.astype(f16), NC_, NC_),
        ow_b=_blocked(o_w.T.astype(f16), NC_, NC_),
        ff1w_b=_blocked(ff1_w.T.astype(f16), NI, NC_),
        ff2w_b=_blocked(ff2_w.T.astype(f16), NC_, NI),
        wv_p=_chunked(wv_t, NC_),
        pos=_chunked(pos_emb[0][:, 768:1024].astype(f16), NC_),
        ob=_chunked(o_b.reshape(D, 1).astype(np.float32), NC_)[:, :, 0],
        f1b=_chunked(ff1_b.reshape(DI, 1).astype(np.float32), NI)[:, :, 0],
        f2b=_chunked(ff2_b.reshape(D, 1).astype(np.float32), NC_)[:, :, 0],
    )
    shared = {k: c(v) for k, v in shared.items()}
    bw = r_w_bias.reshape(D, 1).astype(np.float32)
    br = r_r_bias.reshape(D, 1).astype(np.float32)

    in_maps = []
    for core in range(8):
        b, g = core // 2, core % 2
        q0 = QC * g
        kw0 = q0 + 256
        cat = np.concatenate([z0[b], z1ss[b]], axis=1)
        uq = uss[b, 0:D, 512 + q0:512 + q0 + QC]
        m = dict(shared)
        m.update(
            xq=_chunked(z1ss[b][:, q0:q0 + QC].astype(f16), NC_),
            xkv=_chunked(cat[:, kw0:kw0 + KW].astype(f16), NC_),
            uqw=_chunked((uq + bw).astype(f16), NC_),
            uqr=_chunked((uq + br).astype(f16), NC_),
            uk=_chunked(uss[b, D:2 * D, kw0:kw0 + KW].astype(f16), NC_),
            uvt=_chunked(uss[b, 2 * D:3 * D, kw0:kw0 + KW].T.astype(f16), 4),
        )
        in_maps.append(m)
    return in_maps


def _get_nc():
    if "nc" not in _CACHE:
        _CACHE["nc"] = build_nc()
    return _CACHE["nc"]


def run(in_maps, trace=False, **kw):
    return run_bass_kernel_spmd(_get_nc(), in_maps, core_ids=list(range(8)),
                                trace=trace, **kw)


def kernel(**inputs):
    inputs = {k: np.asarray(v) for k, v in inputs.items()}
    in_maps = _prep_inputs(**inputs)
    res = run(in_maps)
    bsz, qlen = 4, 512
    full = np.empty((bsz, D, qlen), np.float32)
    for core in range(8):
        b, g = core // 2, core % 2
        full[b][:, QC * g:QC * (g + 1)] = res.results[core]["out"].astype(
            np.float32)
    return full


# revision 19
# speedup vs baseline: 1.5241x; 1.5241x over previous
"""Trainium2 Bass kernel for nn_DEQTransformerLM (Transformer-XL style DEQ layer).

Sharding: local-window attention (window 256, qlen 512, mlen 512) makes the
layer fully token-parallel: 8 cores = 4 batches x 2 query-halves of 256
queries. Each core runs qkv projections, relative attention, o-proj+LN and the
FF block for its 256 tokens. No collectives; weights replicated (fp16).

Attention inner loop (per head h, query-chunk c of 128 queries):
  - BD_raw rows are written to DRAM scratch as NEG-padded 513-wide rows and
    read back as a 512-stride view, landing BD_raw[q, r] at score column
    q+1+r with NEG (additive mask) elsewhere - rel_shift + mask in one DMA.
  - score = AC (PSUM) + shifted-BD; EXP on the scalar engine emits the
    softmax denominator per query via accum_out for free; denominators for
    the 4 (c, head) units of a pair batch into one [128,4] reciprocal
    (partition-parallel, fast); probs are normalized pre-transpose on the
    otherwise-idle gpsimd engine.
  - normalized fp16 probs are transposed on the PE (fp16 identity, 1
    cycle/row) into one [128,384] fp16 PSUM tile per unit, then a single
    vector/scalar cast moves them to SBUF for the AV matmuls. No ones/bcast
    matmuls, no [1,256] reciprocals, no fp32 transposes.

Matmul operands fp16 (peak PE rate), accumulation fp32. Layernorm broadcasts
stats to 128 partitions BEFORE sqrt/reciprocal so those run wide.
"""

import contextlib
import os

import numpy as np

import concourse.bacc as bacc
import concourse.mybir as mybir
import concourse.tile as tile
from concourse.bass_utils import run_bass_kernel_spmd
from concourse.masks import make_identity

FP16 = mybir.dt.float16
FP32 = mybir.dt.float32
AF = mybir.ActivationFunctionType
ALU = mybir.AluOpType

D = 1024          # d_model
NH = 16           # heads
DH = 64           # d_head
QC = 256          # queries per core
KW = 512          # key window per core
RW = 256          # relative positions used (cols 768:1024 of r_head_k)
DI = 4096         # d_inner
NC_ = D // 128    # 8 d_model chunks
NI = DI // 128    # 32 d_inner chunks

SCR_ROW = 513     # padded scratch row width (BD rel-shift trick)
SCR_HEAD = QC * SCR_ROW
NEG = -30000.0    # additive mask value (fp16-safe)

_CACHE = {}
PHASE = int(os.environ.get("DEQ_PHASE", "5"))
L_BD = int(os.environ.get("DEQ_L_BD", "2"))    # bd stage lookahead (pairs)


def build_nc():
    nc = bacc.Bacc("TRN2", target_bir_lowering=False, debug=False)
    P = nc.declare_dram_parameter

    # weights pre-blocked on host as [mc, 128, kc*128] (lhsT column blocks,
    # partition-major) so each block load is one contiguous 2KB-row DMA
    wq_b = P("wq_b", [NC_, 128, D], FP16, isOutput=False)
    wk_b = P("wk_b", [NC_, 128, D], FP16, isOutput=False)
    rw_b = P("rw_b", [NC_, 128, D], FP16, isOutput=False)
    ow_b = P("ow_b", [NC_, 128, D], FP16, isOutput=False)
    ff1w_b = P("ff1w_b", [NI, 128, D], FP16, isOutput=False)
    ff2w_b = P("ff2w_b", [NC_, 128, DI], FP16, isOutput=False)
    wv_p = P("wv_p", [128, NC_, D], FP16, isOutput=False)   # Wv.T row-chunks

    # activations pre-arranged to [128, chunk, cols]
    xq = P("xq", [128, NC_, QC], FP16, isOutput=False)
    xkv = P("xkv", [128, NC_, KW], FP16, isOutput=False)
    uqw = P("uqw", [128, NC_, QC], FP16, isOutput=False)
    uqr = P("uqr", [128, NC_, QC], FP16, isOutput=False)
    uk = P("uk", [128, NC_, KW], FP16, isOutput=False)
    uvt = P("uvt", [128, 4, D], FP16, isOutput=False)
    pos = P("pos", [128, NC_, RW], FP16, isOutput=False)
    ob = P("ob", [128, NC_], FP32, isOutput=False)
    f1b = P("f1b", [128, NI], FP32, isOutput=False)
    f2b = P("f2b", [128, NC_], FP32, isOutput=False)

    out = P("out", [D, QC], FP16, isOutput=True)

    scr = nc.dram_tensor("scr", [NH * SCR_HEAD], FP16)

    with contextlib.ExitStack() as _es:
        tc = _es.enter_context(tile.TileContext(nc))
        pool = lambda **kw: _es.enter_context(tc.tile_pool(**kw))
        single = pool(name="single", bufs=1)
        wblk_pool = pool(name="wblk", bufs=8)
        ff1_pool = pool(name="ff1p", bufs=6)
        ff2_pool = pool(name="ff2p", bufs=2)
        bdpad_pool = pool(name="bdpad", bufs=4)
        bdm_pool = pool(name="bdm", bufs=12)
        sc_pool = pool(name="scp", bufs=4)
        pr_pool = pool(name="prp", bufs=4)
        prn_pool = pool(name="prn", bufs=6)
        den_pool = pool(name="den", bufs=3)
        pt_pool = pool(name="ptp", bufs=8)
        rows_pool = pool(name="rows", bufs=1)
        bc_pool = pool(name="bcp", bufs=1)
        out_pool = pool(name="outp", bufs=4)
        mm_ps = pool(name="mm_ps", bufs=2, space="PSUM")
        ac_ps_pool = pool(name="ac_ps", bufs=2, space="PSUM")
        bd_ps_pool = pool(name="bd_ps", bufs=1, space="PSUM")
        tp_ps_pool = pool(name="tp_ps", bufs=2, space="PSUM")
        av_ps_pool = pool(name="av_ps", bufs=1, space="PSUM")
        if True:
            # -------- resident tiles (loads split per chunk for fine deps) ---
            xkv_sb = single.tile([128, NC_, KW], FP16, tag="xkv")
            uk_sb = single.tile([128, NC_, KW], FP16, tag="uk")
            xq_sb = single.tile([128, NC_, QC], FP16, tag="xq")
            uqw_sb = single.tile([128, NC_, QC], FP16, tag="uqw")
            uqr_sb = single.tile([128, NC_, QC], FP16, tag="uqr")
            pos_sb = single.tile([128, NC_, RW], FP16, tag="pos")
            wv_sb = single.tile([128, NC_, D], FP16, tag="wv")
            uvt_sb = single.tile([128, 4, D], FP16, tag="uvt")

            def wload(param, mc):
                t = wblk_pool.tile([128, D], FP16, tag="wblk")
                nc.sync.dma_start(out=t, in_=param[:][mc])
                return t

            # interleave xkv chunks with wk blocks so K proj starts early
            wk_tiles = []
            for kc in range(NC_):
                nc.sync.dma_start(out=xkv_sb[:, kc, :], in_=xkv[:][:, kc, :])
                wk_tiles.append(wload(wk_b, kc))
                nc.scalar.dma_start(out=uk_sb[:, kc, :], in_=uk[:][:, kc, :])
            for kc in range(NC_):
                nc.scalar.dma_start(out=xq_sb[:, kc, :], in_=xq[:][:, kc, :])
                nc.scalar.dma_start(out=uqw_sb[:, kc, :], in_=uqw[:][:, kc, :])
                nc.scalar.dma_start(out=uqr_sb[:, kc, :], in_=uqr[:][:, kc, :])
                nc.scalar.dma_start(out=pos_sb[:, kc, :], in_=pos[:][:, kc, :])
            for kc in range(NC_):
                nc.scalar.dma_start(out=wv_sb[:, kc, :], in_=wv_p[:][:, kc, :])
            for jb in range(4):
                nc.scalar.dma_start(out=uvt_sb[:, jb, :], in_=uvt[:][:, jb, :])

            ones_k = single.tile([128, 1], FP16, tag="ones_k")
            nc.vector.memset(ones_k, 1.0)
            ones_m = single.tile([1, 128], FP16, tag="ones_m")
            nc.vector.memset(ones_m, 1.0)
            negc_sb = single.tile([128, 1], FP32, tag="negc")
            nc.vector.memset(negc_sb, -12.5)
            eps_row = single.tile([1, QC], FP32, tag="eps_row")
            nc.vector.memset(eps_row, 1e-5)
            ident = single.tile([128, 128], FP16, tag="ident")
            make_identity(nc, ident)

            ob_sb = single.tile([128, NC_], FP32, tag="ob")
            nc.gpsimd.dma_start(out=ob_sb, in_=ob[:])
            f1b_sb = single.tile([128, NI], FP32, tag="f1b")
            nc.gpsimd.dma_start(out=f1b_sb, in_=f1b[:])
            f2b_sb = single.tile([128, NC_], FP32, tag="f2b")
            nc.gpsimd.dma_start(out=f2b_sb, in_=f2b[:])

            k_sb = single.tile([128, NC_, KW], FP16, tag="k_sb")
            vT_sb = single.tile([128, 4, D], FP16, tag="vT_sb")
            rwq_sb = single.tile([128, NC_, QC], FP16, tag="rwq")
            rrq_sb = single.tile([128, NC_, QC], FP16, tag="rrq")
            r_sb = single.tile([128, NC_, RW], FP16, tag="r_sb")
            attn_sb = single.tile([128, NC_, QC], FP16, tag="attn")
            x_sb = single.tile([128, NC_, QC], FP16, tag="x_sb")
            h_sb = single.tile([128, NI, QC], FP16, tag="h_sb")

            # ---------------- K projection ----------------
            for mc in range(NC_ if PHASE >= 1 else 0):
                wt = wk_tiles[mc]
                ps = mm_ps.tile([128, KW], FP32, tag="mm")
                for kc in range(NC_):
                    nc.tensor.matmul(ps, wt[:, 128 * kc:128 * (kc + 1)],
                                     xkv_sb[:, kc, :],
                                     start=(kc == 0), stop=(kc == NC_ - 1))
                nc.vector.tensor_tensor(out=k_sb[:, mc, :], in0=ps,
                                        in1=uk_sb[:, mc, :], op=ALU.add)

            # ---------------- Q projection ----------------
            for mc in range(NC_ if PHASE >= 1 else 0):
                wt = wload(wq_b, mc)
                ps = mm_ps.tile([128, QC], FP32, tag="mm")
                for kc in range(NC_):
                    nc.tensor.matmul(ps, wt[:, 128 * kc:128 * (kc + 1)],
                                     xq_sb[:, kc, :],
                                     start=(kc == 0), stop=(kc == NC_ - 1))
                nc.vector.tensor_tensor(out=rwq_sb[:, mc, :], in0=ps,
                                        in1=uqw_sb[:, mc, :], op=ALU.add)
                nc.vector.tensor_tensor(out=rrq_sb[:, mc, :], in0=ps,
                                        in1=uqr_sb[:, mc, :], op=ALU.add)

            # ---------------- R projection ----------------
            for mc in range(NC_ if PHASE >= 1 else 0):
                wt = wload(rw_b, mc)
                ps = mm_ps.tile([128, RW], FP32, tag="mm")
                for kc in range(NC_):
                    nc.tensor.matmul(ps, wt[:, 128 * kc:128 * (kc + 1)],
                                     pos_sb[:, kc, :],
                                     start=(kc == 0), stop=(kc == NC_ - 1))
                nc.vector.tensor_copy(r_sb[:, mc, :], ps)

            # ---------------- V projection (transposed output) --------------
            for jb in range(4 if PHASE >= 1 else 0):
                for dh in range(2):
                    ps = mm_ps.tile([128, 512], FP32, tag="mm")
                    for kc in range(NC_):
                        nc.tensor.matmul(
                            ps,
                            xkv_sb[:, kc, 128 * jb:128 * (jb + 1)],
                            wv_sb[:, kc, 512 * dh:512 * (dh + 1)],
                            start=(kc == 0), stop=(kc == NC_ - 1))
                    nc.vector.tensor_tensor(
                        out=vT_sb[:, jb, 512 * dh:512 * (dh + 1)], in0=ps,
                        in1=uvt_sb[:, jb, 512 * dh:512 * (dh + 1)], op=ALU.add)

            # ------- attention, software-pipelined over head PAIRS ----------
            # pair pi = heads (2*pi, 2*pi+1): even head in partition rows
            # 0:64, odd in 64:128 of d-chunk hc=pi.
            scrap = scr[:]
            bdm_tiles = {}   # pi -> {(c, sub): shifted-BD tile [128, 384]}
            pt_tiles = {}    # pi -> {(c, sub): transposed probs [128, 384]}

            def bd_stage(pi):
                # BD matmuls + padded scratch writes + shifted read issue
                hc = pi
                bdm_tiles[pi] = {}
                for c in range(2):
                    for sub in range(2):
                        h = 2 * pi + sub
                        hsl = slice(64 * sub, 64 * sub + 64)
                        bdp = bd_ps_pool.tile([128, RW], FP32, tag="bd")
                        nc.tensor.matmul(
                            bdp, rrq_sb[hsl, hc, 128 * c:128 * (c + 1)],
                            r_sb[hsl, hc, :], start=True, stop=True)
                        pad = bdpad_pool.tile([128, SCR_ROW], FP16, tag="pad")
                        if pi == 0:
                            nc.gpsimd.memset(pad, NEG)
                        if sub == 0:
                            nc.vector.tensor_copy(pad[:, 1:257], bdp)
                        else:
                            nc.scalar.copy(pad[:, 1:257], bdp)
                        wbase = h * SCR_HEAD + SCR_ROW * 128 * c
                        nc.sync.dma_start(
                            out=scrap[wbase:wbase + 128 * SCR_ROW].rearrange(
                                "(p f) -> p f", f=SCR_ROW),
                            in_=pad)
                        rbase = h * SCR_HEAD + KW * 128 * c + 128 * c
                        bdm = bdm_pool.tile([128, 384], FP16, tag="bdm")
                        nc.scalar.dma_start(
                            out=bdm,
                            in_=scrap[rbase:rbase + 128 * KW].rearrange(
                                "(p f) -> p f", f=KW)[:, 0:384])
                        bdm_tiles[pi][(c, sub)] = bdm

            def sm_stage(pi):
                hc = pi
                pt_tiles[pi] = {}
                den4 = den_pool.tile([128, 4], FP32, tag="den")
                prns = {}
                for u, (c, sub) in enumerate(
                        (c, s) for c in range(2) for s in range(2)):
                    hsl = slice(64 * sub, 64 * sub + 64)
                    acp = ac_ps_pool.tile([128, 384], FP32, tag="ac")
                    nc.tensor.matmul(
                        acp, rwq_sb[hsl, hc, 128 * c:128 * (c + 1)],
                        k_sb[hsl, hc, 128 * c:128 * c + 384],
                        start=True, stop=True)
                    sc = sc_pool.tile([128, 384], FP16, tag="sc")
                    nc.vector.tensor_tensor(
                        out=sc, in0=acp, in1=bdm_tiles[pi][(c, sub)],
                        op=ALU.add)
                    pr = pr_pool.tile([128, 384], FP16, tag="pr")
                    nc.scalar.activation(out=pr, in_=sc, func=AF.Exp,
                                         scale=0.125, bias=negc_sb,
                                         accum_out=den4[:, u:u + 1])
                    prns[(c, sub)] = pr
                nc.vector.reciprocal(out=den4, in_=den4)
                for u, (c, sub) in enumerate(
                        (c, s) for c in range(2) for s in range(2)):
                    prn = prn_pool.tile([128, 384], FP16, tag="prn")
                    nc.vector.tensor_scalar_mul(prn, prns[(c, sub)],
                                                den4[:, u:u + 1])
                    tp = tp_ps_pool.tile([128, 384], FP16, tag="tp")
                    for t in range(3):
                        nc.tensor.transpose(
                            tp[:, 128 * t:128 * (t + 1)],
                            prn[:, 128 * t:128 * (t + 1)], ident)
                    pt = pt_pool.tile([128, 384], FP16, tag="pt")
                    if u % 2 == 0:
                        nc.vector.tensor_copy(pt, tp)
                    else:
                        nc.scalar.copy(pt, tp)
                    pt_tiles[pi][(c, sub)] = pt
                del bdm_tiles[pi]

            def av_stage(pi):
                hc = pi
                for sub in range(2):
                    h = 2 * pi + sub
                    avp = av_ps_pool.tile([64, 256], FP32, tag="av")
                    for c in range(2):
                        ptt = pt_tiles[pi][(c, sub)]
                        for t in range(3):
                            kb = t + c
                            nc.tensor.matmul(
                                avp[:, 128 * c:128 * (c + 1)],
                                vT_sb[:, kb, 64 * h:64 * h + 64],
                                ptt[:, 128 * t:128 * (t + 1)],
                                start=(t == 0), stop=(t == 2))
                    nc.vector.tensor_copy(
                        attn_sb[64 * sub:64 * sub + 64, hc, :], avp)
                del pt_tiles[pi]

            NP_ = NH // 2
            if PHASE >= 2:
                for pi in range(min(L_BD, NP_)):
                    bd_stage(pi)
                sm_stage(0)
                for pi in range(NP_):
                    if pi + L_BD < NP_:
                        bd_stage(pi + L_BD)
                    if pi + 1 < NP_:
                        sm_stage(pi + 1)
                    av_stage(pi)

            # ---------------- O projection + LN1 ----------------
            # stack rows: cols 0:256 = attn_out + residual, 256:512 = square
            stack1 = single.tile([128, NC_, 512], FP16, tag="stack")
            for mc in range(NC_ if PHASE >= 3 else 0):
                wt = wload(ow_b, mc)
                ps = mm_ps.tile([128, QC], FP32, tag="mm")
                for kc in range(NC_):
                    nc.tensor.matmul(ps, wt[:, 128 * kc:128 * (kc + 1)],
                                     attn_sb[:, kc, :],
                                     start=(kc == 0), stop=(kc == NC_ - 1))
                nc.vector.scalar_tensor_tensor(
                    out=stack1[:, mc, 0:256], in0=ps, scalar=ob_sb[:, mc:mc + 1],
                    in1=xq_sb[:, mc, :], op0=ALU.add, op1=ALU.add)
                nc.vector.tensor_tensor(
                    out=stack1[:, mc, 256:512], in0=stack1[:, mc, 0:256],
                    in1=stack1[:, mc, 0:256], op=ALU.mult)

            def layernorm(stack, xout):
                # red [1,512] = (sum x | sum x^2); broadcast FIRST, then all
                # per-token math runs on 128 partitions.
                redt = mm_ps.tile([128, 512], FP32, tag="mm")
                red = redt[0:1, :]
                for mc in range(NC_):
                    nc.tensor.matmul(red, ones_k, stack[:, mc, :],
                                     start=(mc == 0), stop=(mc == NC_ - 1))
                row = rows_pool.tile([1, 512], FP32, tag="row")
                nc.vector.tensor_scalar_mul(row, red, 1.0 / D)  # mean | E[x^2]
                msq = rows_pool.tile([1, QC], FP32, tag="msq")
                nc.vector.tensor_tensor(out=msq, in0=row[0:1, 0:256],
                                        in1=row[0:1, 0:256], op=ALU.mult)
                # var + eps in cols 256:512
                nc.vector.tensor_tensor(out=row[0:1, 256:512],
                                        in0=row[0:1, 256:512], in1=msq,
                                        op=ALU.subtract)
                nc.vector.tensor_tensor(out=row[0:1, 256:512],
                                        in0=row[0:1, 256:512], in1=eps_row,
                                        op=ALU.add)
                row16 = rows_pool.tile([1, 512], FP16, tag="row16")
                nc.vector.tensor_copy(row16, row)
                bps = mm_ps.tile([128, 512], FP32, tag="mm")
                nc.tensor.matmul(bps, ones_m, row16, start=True, stop=True)
                std = bc_pool.tile([128, QC], FP32, tag="std")
                nc.scalar.activation(out=std, in_=bps[:, 256:512],
                                     func=AF.Sqrt)
                nc.vector.reciprocal(out=std, in_=std)
                mean = bc_pool.tile([128, QC], FP16, tag="mean")
                nc.vector.tensor_copy(mean, bps[:, 0:256])
                rstd16 = bc_pool.tile([128, QC], FP16, tag="rstd16")
                nc.vector.tensor_copy(rstd16, std)
                for mc in range(NC_):
                    xo = xout(mc)
                    nc.vector.tensor_tensor(
                        out=xo, in0=stack[:, mc, 0:256],
                        in1=mean, op=ALU.subtract)
                    nc.vector.tensor_tensor(out=xo, in0=xo, in1=rstd16,
                                            op=ALU.mult)

            if PHASE >= 3:
                layernorm(stack1, lambda mc: x_sb[:, mc, :])

            # ---------------- FF1 ----------------
            for mc in range(NI if PHASE >= 4 else 0):
                wt = ff1_pool.tile([128, D], FP16, tag="ff1")
                (nc.sync if mc % 2 == 0 else nc.scalar).dma_start(
                    out=wt, in_=ff1w_b[:][mc])
                ps = mm_ps.tile([128, QC], FP32, tag="mm")
                for kc in range(NC_):
                    nc.tensor.matmul(ps, wt[:, 128 * kc:128 * (kc + 1)],
                                     x_sb[:, kc, :],
                                     start=(kc == 0), stop=(kc == NC_ - 1))
                nc.scalar.activation(out=h_sb[:, mc, :], in_=ps, func=AF.Relu,
                                     bias=f1b_sb[:, mc:mc + 1], scale=1.0)

            # ---------------- FF2 + LN2 + output ----------------
            stack2 = single.tile([128, NC_, 512], FP16, tag="stack2")
            for mc in range(NC_ if PHASE >= 5 else 0):
                wt = ff2_pool.tile([128, DI], FP16, tag="ff2")
                (nc.sync if mc % 2 == 0 else nc.scalar).dma_start(
                    out=wt, in_=ff2w_b[:][mc])
                ps = mm_ps.tile([128, QC], FP32, tag="mm")
                for kc in range(NI):
                    nc.tensor.matmul(ps, wt[:, 128 * kc:128 * (kc + 1)],
                                     h_sb[:, kc, :],
                                     start=(kc == 0), stop=(kc == NI - 1))
                nc.vector.scalar_tensor_tensor(
                    out=stack2[:, mc, 0:256], in0=ps, scalar=f2b_sb[:, mc:mc + 1],
                    in1=x_sb[:, mc, :], op0=ALU.add, op1=ALU.add)
                nc.vector.tensor_tensor(
                    out=stack2[:, mc, 256:512], in0=stack2[:, mc, 0:256],
                    in1=stack2[:, mc, 0:256], op=ALU.mult)

            out_tiles = {}

            def out_tile(mc):
                t = out_pool.tile([128, QC], FP16, tag="out")
                out_tiles[mc] = t
                return t

            if PHASE >= 5:
                layernorm(stack2, out_tile)
            else:
                for mc in range(NC_):
                    t = out_tile(mc)
                    nc.vector.memset(t, 0.0)
            for mc in range(NC_):
                (nc.sync if mc % 2 == 0 else nc.scalar).dma_start(
                    out=out[:][128 * mc:128 * (mc + 1), :], in_=out_tiles[mc])

    nc.compile()
    return nc

def _chunked(a, n):
    # [n*128, w] -> [128, n, w] partition-major
    w = a.shape[1]
    return np.ascontiguousarray(a.reshape(n, 128, w).transpose(1, 0, 2))


def _blocked(wt, nmc, nkc):
    # wt [K, M] (transposed weight) -> [mc, 128, kc*128] lhsT column blocks
    return np.ascontiguousarray(
        wt.reshape(nkc, 128, nmc, 128).transpose(2, 1, 0, 3).reshape(
            nmc, 128, nkc * 128))


def _prep_inputs(z1ss, uss, z0, pos_emb, qkv_w, r_w, r_w_bias, r_r_bias, o_w,
                 o_b, ff1_w, ff1_b, ff2_w, ff2_b):
    f16 = np.float16
    c = np.ascontiguousarray

    wq_t = qkv_w[0:D].T.astype(f16)
    wk_t = qkv_w[D:2 * D].T.astype(f16)
    wv_t = qkv_w[2 * D:3 * D].T.astype(f16)
    shared = dict(
        wq_b=_blocked(wq_t, NC_, NC_), wk_b=_blocked(wk_t, NC_, NC_),
        rw_b=_blocked(r_w.T.astype(f16), NC_, NC_),
        ow_b=_blocked(o_w.T.astype(f16), NC_, NC_),
        ff1w_b=_blocked(ff1_w.T.astype(f16), NI, NC_),
        ff2w_b=_blocked(ff2_w.T.astype(f16), NC_, NI),
        wv_p=_chunked(wv_t, NC_),
        pos=_chunked(pos_emb[0][:, 768:1024].astype(f16), NC_),
        ob=_chunked(o_b.reshape(D, 1).astype(np.float32), NC_)[:, :, 0],
        f1b=_chunked(ff1_b.reshape(DI, 1).astype(np.float32), NI)[:, :, 0],
        f2b=_chunked(ff2_b.reshape(D, 1).astype(np.float32), NC_)[:, :, 0],
    )
    shared = {k: c(v) for k, v in shared.items()}
    bw = r_w_bias.reshape(D, 1).astype(np.float32)
    br = r_r_bias.reshape(D, 1).astype(np.float32)

    in_maps = []
    for core in range(8):
        b, g = core // 2, core % 2
        q0 = QC * g
        kw0 = q0 + 256
        cat = np.concatenate([z0[b], z1ss[b]], axis=1)
        uq = uss[b, 0:D, 512 + q0:512 + q0 + QC]
        m = dict(shared)
        m.update(
            xq=_chunked(z1ss[b][:, q0:q0 + QC].astype(f16), NC_),
            xkv=_chunked(cat[:, kw0:kw0 + KW].astype(f16), NC_),
            uqw=_chunked((uq + bw).astype(f16), NC_),
            uqr=_chunked((uq + br).astype(f16), NC_),
            uk=_chunked(uss[b, D:2 * D, kw0:kw0 + KW].astype(f16), NC_),
            uvt=_chunked(uss[b, 2 * D:3 * D, kw0:kw0 + KW].T.astype(f16), 4),
        )
        in_maps.append(m)
    return in_maps


def _get_nc():
    if "nc" not in _CACHE:
        _CACHE["nc"] = build_nc()
    return _CACHE["nc"]


def run(in_maps, trace=False, **kw):
    return run_bass_kernel_spmd(_get_nc(), in_maps, core_ids=list(range(8)),
                                trace=trace, **kw)


def kernel(**inputs):
    inputs = {k: np.asarray(v) for k, v in inputs.items()}
    in_maps = _prep_inputs(**inputs)
    res = run(in_maps)
    bsz, qlen = 4, 512
    full = np.empty((bsz, D, qlen), np.float32)
    for core in range(8):
        b, g = core // 2, core % 2
        full[b][:, QC * g:QC * (g + 1)] = res.results[core]["out"].astype(
            np.float32)
    return full
